# revision 2
# baseline (speedup 1.0000x reference)
"""Swin-style windowed MHA on 8 Trainium2 cores — v2 (pipelined rewrite).

Differences vs baseline:
  - 3-stage window pipeline on PE: scores(W+1) | AV+drain(W) | proj(W-1),
    with next group's qkv GEMM bursts as filler -> PE never waits on the
    softmax drain chain, HAM stays warm.
  - normalize drain per head-pair: 2 ACT sums copies [1,196] -> one DVE
    reciprocal [2,196] (batched; baseline burned 644ns single-lane recips)
    -> 2 gpsimd partition-broadcasts [64,196] -> 2 DVE evict-muls.
  - proj is weight-stationary (full 128-col util; baseline had M=68 chunks)
    producing channel-major y in bf16; host transposes back.
  - AV accumulates into ONE psum bank per head-pair ([65, 392], j0/j1 in one
    start/stop envelope) freeing banks: scores 4 + AV 2 + gemm 2 = 8.
  - biases are zero per the problem spec (asserted on host).
"""

import numpy as np
import ml_dtypes

WS = 14
NH = 12
HD = 64
C = 768
N = WS * WS  # 196
NCORES = 8

_BF16 = ml_dtypes.bfloat16
_prog_cache = {}

MC = [(0, 128), (128, 68)]  # token chunks within a 196-token window


def _rel_index(ws):
    coords = np.stack(np.meshgrid(np.arange(ws), np.arange(ws), indexing="ij"))
    cf = coords.reshape(2, -1)
    rel = (cf[:, :, None] - cf[:, None, :]).transpose(1, 2, 0).astype(np.int64)
    rel[..., 0] += ws - 1
    rel[..., 1] += ws - 1
    rel[..., 0] *= 2 * ws - 1
    return rel.sum(-1)


import os

CFG = {
    "recip_psum": os.environ.get("V2_RECIP_PSUM", "1") == "1",
    "y_bf16": os.environ.get("V2_Y_BF16", "1") == "1",
    "av_env": os.environ.get("V2_AV_ENV", "1") == "1",
}


def _build_program(n_win):
    import concourse.bass as bass
    import concourse.mybir as mybir
    import concourse.tile as tile
    from concourse import bacc
    from contextlib import ExitStack

    assert n_win % 4 == 0
    n_grp = n_win // 4
    n_tok = n_win * N

    BF = mybir.dt.bfloat16
    F32 = mybir.dt.float32
    AF = mybir.ActivationFunctionType

    nc = bacc.Bacc("TRN2", target_bir_lowering=False, debug=False,
                   num_devices=NCORES)

    x = nc.dram_tensor("x", [n_tok, C], BF, kind="ExternalInput")
    wqkvT = nc.dram_tensor("wqkvT", [C, 3 * C], BF, kind="ExternalInput")
    wpT = nc.dram_tensor("wpT", [C, C], BF, kind="ExternalInput")
    er = nc.dram_tensor("er", [N, NH * N], BF, kind="ExternalInput")
    YDT = BF if CFG["y_bf16"] else F32
    y = nc.dram_tensor("y", [C, n_tok], YDT, kind="ExternalOutput")

    with ExitStack() as ctx:
        tc = ctx.enter_context(tile.TileContext(nc))
        consts = ctx.enter_context(tc.tile_pool(name="consts", bufs=1))
        grp = ctx.enter_context(tc.tile_pool(name="grp", bufs=2))
        win = ctx.enter_context(tc.tile_pool(name="win", bufs=2))
        # PSUM: 8 banks total. scores [128,1024] (2 banks) x bufs=2 = 4;
        # AV [65,392] (1 bank) x bufs=2 = 2; gemm [128,392] (1 bank) x2 = 2.
        psc = ctx.enter_context(tc.tile_pool(name="psc", bufs=2, space="PSUM"))
        psa = ctx.enter_context(tc.tile_pool(
            name="psa", bufs=(2 if CFG["av_env"] else 1), space="PSUM"))
        psg = ctx.enter_context(tc.tile_pool(name="psg", bufs=2, space="PSUM"))

        # ---- constants -------------------------------------------------
        wq = []
        for ic in range(6):
            t = consts.tile([128, 3 * C], BF, tag=f"wq{ic}", name=f"wq{ic}")
            nc.sync.dma_start(out=t, in_=wqkvT[ic * 128:(ic + 1) * 128, :])
            wq.append(t)
        wp = []
        for ic in range(6):
            t = consts.tile([128, C], BF, tag=f"wp{ic}", name=f"wp{ic}")
            nc.sync.dma_start(out=t, in_=wpT[ic * 128:(ic + 1) * 128, :])
            wp.append(t)
        er_t = []
        for mci, (mo, msz) in enumerate(MC):
            t = consts.tile([msz, NH * N], BF, tag=f"er{mci}", name=f"er{mci}")
            nc.sync.dma_start(out=t, in_=er[mo:mo + msz, :])
            er_t.append(t)

        # ---- per-group state -------------------------------------------
        # group g tiles are alloc'd via pools with bufs=2 (g, g+1 rotate)
        class G:
            __slots__ = ("xT", "qk", "v")

        def alloc_group():
            s = G()
            s.xT = [grp.tile([128, 4 * N], BF, tag=f"xT{ic}", name=f"xT{ic}")
                    for ic in range(6)]
            s.qk = [grp.tile([128, 4 * N], BF, tag=f"qk{oc}", name=f"qk{oc}")
                    for oc in range(12)]
            s.v = {}
            for w4 in range(4):
                for mci, (mo, msz) in enumerate(MC):
                    s.v[(w4, mci)] = grp.tile(
                        [128, NH * 65], BF, tag=f"v{w4}_{mci}",
                        name=f"v{w4}_{mci}")
            return s

        def emit_xT_dma(s, g):
            t0 = g * 4 * N
            for ic in range(6):
                nc.sync.dma_start(
                    out=s.xT[ic],
                    in_=x[t0:t0 + 4 * N, ic * 128:(ic + 1) * 128],
                    transpose=True)

        # alternate psum evictions between DVE and ACT so neither queue
        # gates psum-bank reuse (PE stalls tracked to evicts stuck in FIFO)
        ev_tog = [0]

        def evict_copy(out, in_):
            ev_tog[0] ^= 1
            if ev_tog[0]:
                nc.vector.tensor_copy(out, in_)
            else:
                nc.scalar.activation(out, in_, AF.Copy)

        # ---- qkv bursts for a group (producer side) --------------------
        def qkv_burst_list(s):
            """List of thunks; each emits ~6 PE matmuls + an evict."""
            thunks = []

            def mk_qk(oc, sc):
                def f():
                    ps = psg.tile([128, 392], F32, tag="psg", name="psg")
                    col = oc * 128 if oc < 6 else 768 + (oc - 6) * 128
                    for ic in range(6):
                        nc.tensor.matmul(
                            ps,
                            wq[ic][:, col:col + 128],
                            s.xT[ic][:, sc * 392:(sc + 1) * 392],
                            start=(ic == 0), stop=(ic == 5))
                    evict_copy(s.qk[oc][:, sc * 392:(sc + 1) * 392], ps)
                return f

            for oc in range(12):
                for sc in range(2):
                    thunks.append(mk_qk(oc, sc))

            def mk_v(w4, mci, half):
                mo, msz = MC[mci]

                def f():
                    vt = s.v[(w4, mci)]
                    vr = vt.rearrange("p (h e) -> p h e", e=65)
                    ps = psg.tile([128, 392], F32, tag="psg", name="psg")
                    for ic in range(6):
                        nc.tensor.matmul(
                            ps[:msz, 0:384],
                            s.xT[ic][:, w4 * N + mo: w4 * N + mo + msz],
                            wq[ic][:, 1536 + half * 384: 1536 + (half + 1) * 384],
                            start=(ic == 0), stop=(ic == 5))
                    evict_copy(
                        vr[:msz, half * 6:(half + 1) * 6, 0:64],
                        ps[:msz, 0:384].rearrange("p (h e) -> p h e", e=64))
                    nc.vector.memset(vr[:msz, half * 6:(half + 1) * 6, 64:65], 1.0)
                return f

            for w4 in range(4):
                for mci in range(2):
                    for half in range(2):
                        thunks.append(mk_v(w4, mci, half))
            return thunks  # 24 + 16 = 40

        # ---- per-window state ------------------------------------------
        class Wst:
            __slots__ = ("ex", "attn", "rr", "ps_av", "g", "w4")

        def alloc_window(g, w4):
            ws_ = Wst()
            ws_.g, ws_.w4 = g, w4
            ws_.ex = [win.tile([msz, NH * N], BF, tag=f"ex{mci}",
                               name=f"ex{mci}")
                      for mci, (mo, msz) in enumerate(MC)]
            ws_.attn = [win.tile([msz, NH * N], BF, tag=f"attn{mci}",
                                 name=f"attn{mci}")
                        for mci, (mo, msz) in enumerate(MC)]
            ws_.rr = [None] * 6
            ws_.ps_av = [None] * 6
            return ws_

        def emit_score_pair(ws_, gs, p):
            """p = hg*2 + mci: one [128,1024] psum, heads 2hg,2hg+1 packed."""
            hg, mci = p // 2, p % 2
            mo, msz = MC[mci]
            w0 = ws_.w4 * N
            ps = psc.tile([128, 1024], F32, tag="psc", name="psc")
            for j in range(2):
                nc.tensor.matmul(
                    ps[:msz, j * 512:j * 512 + N],
                    gs.qk[6 + hg][j * 64:j * 64 + 64, w0 + mo: w0 + mo + msz],
                    gs.qk[hg][j * 64:j * 64 + 64, w0:w0 + N],
                    start=True, stop=True)
            # exp -> ex[mci][:, 2hg:2hg+2, :]
            nc.scalar.activation(
                ws_.ex[mci].rearrange("p (h n) -> p h n", n=N)
                    [:, 2 * hg:2 * hg + 2, :],
                ps[:msz].rearrange("p (j n) -> p j n", n=512)[:, :, 0:N],
                AF.Exp)

        def emit_ermul(ws_, half, mci):
            c0 = half * 6 * N
            c1 = (half + 1) * 6 * N
            nc.vector.tensor_mul(
                ws_.attn[mci][:, c0:c1], ws_.ex[mci][:, c0:c1],
                er_t[mci][:, c0:c1])

        def emit_av(ws_, gs, k):
            """AV for head pair k: one [65,392] psum bank, single start/stop
            envelope covering j=0,1 x mci accumulation (av_env), or a 2-bank
            [65,1024] tile with per-j groups (fallback)."""
            if CFG["av_env"]:
                ps = psa.tile([65, 392], F32, tag="psa", name="psa")
                ws_.ps_av[k] = ps
                first = True
                for j in range(2):
                    h = 2 * k + j
                    for mci, (mo, msz) in enumerate(MC):
                        nc.tensor.matmul(
                            ps[:, j * N:(j + 1) * N],
                            gs.v[(ws_.w4, mci)][:msz, h * 65:(h + 1) * 65],
                            ws_.attn[mci][:, h * N:(h + 1) * N],
                            start=first, stop=(j == 1 and mci == 1),
                            skip_group_check=True)
                        first = False
            else:
                ps2 = psa.tile([65, 1024], F32, tag="psa", name="psa")
                ws_.ps_av[k] = ps2
                for j in range(2):
                    h = 2 * k + j
                    for mci, (mo, msz) in enumerate(MC):
                        nc.tensor.matmul(
                            ps2[:, j * 512:j * 512 + N],
                            gs.v[(ws_.w4, mci)][:msz, h * 65:(h + 1) * 65],
                            ws_.attn[mci][:, h * N:(h + 1) * N],
                            start=(mci == 0), stop=(mci == 1))

        AVS = N if CFG["av_env"] else 512  # column stride between j halves

        def emit_recip(ws_, k):
            """sums (PSUM row 64 of the AV bank) -> SBUF -> 1/sums."""
            sm = win.tile([1, 2 * N], F32, tag="smc", name="smc", bufs=4)
            rr = win.tile([1, 2 * N], F32, tag="rr", name="rr", bufs=4)
            ws_.rr[k] = rr
            src = (ws_.ps_av[k][64:65, :]
                   .rearrange("p (j n) -> p j n", n=AVS)[:, :, 0:N])
            nc.scalar.activation(sm, src, AF.Copy)
            nc.vector.reciprocal_approx_fast(rr, sm)

        def emit_bcast_mul(ws_, k, rrep, aoT):
            """broadcast 1/sums and normalize-evict into aoT (pair tile)."""
            ps = ws_.ps_av[k]
            wpar = ws_.w4 % 2
            # one full-height broadcast of the raw sums [1, 2N] -> [128, 2N];
            # every row then holds both j-halves (HW broadcast ucode starts at
            # partition 0 regardless of out base, so slice sources from 0)
            nc.gpsimd.partition_broadcast(rrep, ws_.rr[k])
            for j in range(2):
                h = 2 * k + j
                nc.vector.tensor_mul(
                    aoT[h // 2][(h % 2) * 64:(h % 2) * 64 + 64,
                                wpar * N:(wpar + 1) * N],
                    ps[0:64, j * AVS:j * AVS + N],
                    rrep[0:64, j * N:(j + 1) * N])

        # ---- proj (weight-stationary, per window-pair) ------------------
        def proj_burst_list(aoT, pair_idx):
            thunks = []

            def mk(oc):
                def f():
                    ps = psg.tile([128, 392], F32, tag="psg", name="psg")
                    for ic in range(6):
                        nc.tensor.matmul(
                            ps,
                            wp[ic][:, oc * 128:(oc + 1) * 128],
                            aoT[ic][:, 0:392],
                            start=(ic == 0), stop=(ic == 5))
                    ysb = win.tile([128, 392], BF, tag="ysb", name="ysb",
                                   bufs=3)
                    evict_copy(ysb, ps)
                    nc.sync.dma_start(
                        out=y[oc * 128:(oc + 1) * 128,
                              pair_idx * 392:(pair_idx + 1) * 392],
                        in_=ysb)
                return f

            for oc in range(6):
                thunks.append(mk(oc))
            return thunks

        # ---- the pipeline ------------------------------------------------
        n_slots = n_win
        cur_g = alloc_group()
        emit_xT_dma(cur_g, 0)
        nxt_g = None
        if n_grp > 1:
            nxt_g = alloc_group()
            emit_xT_dma(nxt_g, 1)

        # prologue: qkv(0) fully, then scores(0)
        for th in qkv_burst_list(cur_g):
            th()

        win_states = {}
        win_states[0] = alloc_window(0, 0)
        for p in range(12):
            emit_score_pair(win_states[0], cur_g, p)
            if p == 5:
                emit_ermul(win_states[0], 0, 0)
                emit_ermul(win_states[0], 0, 1)
        emit_ermul(win_states[0], 1, 0)
        emit_ermul(win_states[0], 1, 1)

        # per-pair aoT and rrep tiles
        aoT_cur = None
        rrep_tiles = {}
        proj_queue = []  # pending proj thunks
        qkv_queue = []   # pending qkv thunks for next group

        groups = [None] * n_grp
        groups[0] = cur_g
        if n_grp > 1:
            groups[1] = nxt_g

        pend_xT = {}  # group -> needs dma emit at slot

        for W in range(n_slots):
            g, w4 = W // 4, W % 4
            gs = groups[g]
            ws_ = win_states[W]

            # window W+1 (may be next group)
            if W + 1 < n_slots:
                g1, w41 = (W + 1) // 4, (W + 1) % 4
                ws_n = alloc_window(g1, w41)
                win_states[W + 1] = ws_n
                gs_n = groups[g1]
            else:
                ws_n = None

            # start-of-group bookkeeping
            if w4 == 0:
                # queue qkv bursts for group g+1 (emitted through this group)
                if g + 1 < n_grp:
                    qkv_queue = list(qkv_burst_list(groups[g + 1]))
                else:
                    qkv_queue = []
            # schedule xT dma for group g+2 near start of group g+1's
            # predecessor (2 slots into group g)
            if w4 == 2 and g + 2 < n_grp:
                groups[g + 2] = alloc_group()
                emit_xT_dma(groups[g + 2], g + 2)

            # pair-level tiles for (W even: allocate)
            if w4 % 2 == 0:
                aoT_cur = [win.tile([128, 392], BF, tag=f"aoT{i}",
                                    name=f"aoT{i}") for i in range(6)]

            # spread remaining qkv bursts evenly over this group's slots
            slots_left = 4 - w4
            quota = [-(-len(qkv_queue) // slots_left)] if qkv_queue else [0]

            def qkv_step(n):
                for _ in range(n):
                    if qkv_queue and quota[0] > 0:
                        qkv_queue.pop(0)()
                        quota[0] -= 1

            # main interleave
            for p in range(12):
                k = p // 2
                if p % 2 == 0:
                    emit_av(ws_, gs, k)
                    if k > 0:
                        emit_recip(ws_, k - 1)
                        rrep_prev = rrep_tiles[(W, k - 1)]
                        emit_bcast_mul(ws_, k - 1, rrep_prev, aoT_cur)
                    rrep_tiles[(W, k)] = win.tile(
                        [128, 2 * N], F32, tag="rrep", name="rrep", bufs=4)
                if ws_n is not None:
                    emit_score_pair(ws_n, gs_n, p)
                    if p == 5:
                        emit_ermul(ws_n, 0, 0)
                        emit_ermul(ws_n, 0, 1)
                    if p == 11:
                        emit_ermul(ws_n, 1, 0)
                        emit_ermul(ws_n, 1, 1)
                qkv_step(1)
                if p in (3, 7, 11):
                    for _ in range(1):
                        if proj_queue:
                            proj_queue.pop(0)()
                qkv_step(1 if p % 2 else 0)

            # finish window W's drain
            emit_recip(ws_, 5)
            emit_bcast_mul(ws_, 5, rrep_tiles[(W, 5)], aoT_cur)

            # after odd windows: queue proj for the pair (W-1, W)
            if w4 % 2 == 1:
                proj_queue.extend(proj_burst_list(aoT_cur, W // 2))

            # cleanup old state
            win_states.pop(W - 1, None)
            for kk in range(6):
                rrep_tiles.pop((W - 1, kk), None)

        # epilogue: remaining proj bursts
        while proj_queue:
            proj_queue.pop(0)()

    nc.compile()
    return nc


def _get_program(n_win):
    if n_win not in _prog_cache:
        _prog_cache[n_win] = _build_program(n_win)
    return _prog_cache[n_win]


def _host_prep(x, qkv_w, q_bias, v_bias, rel_bias_table, proj_w, proj_b, H, W):
    B = x.shape[0]
    nws = H // WS
    xw = (np.asarray(x, np.float32)
          .reshape(B, nws, WS, nws, WS, C)
          .transpose(0, 1, 3, 2, 4, 5)
          .reshape(-1, N, C))

    scale = HD ** -0.5
    wq_s = np.array(qkv_w, np.float32, copy=True)
    wq_s[0:C] *= scale
    wqkvT = np.ascontiguousarray(wq_s.T).astype(_BF16)
    wpT = np.ascontiguousarray(np.asarray(proj_w, np.float32).T).astype(_BF16)

    idx = _rel_index(WS).reshape(-1)
    rpb = np.asarray(rel_bias_table, np.float32)[idx].reshape(N, N, NH)
    er = np.ascontiguousarray(
        np.exp(rpb).transpose(1, 2, 0).reshape(N, NH * N)).astype(_BF16)

    xbf = np.ascontiguousarray(xw.reshape(-1, C)).astype(_BF16)
    return xbf, wqkvT, wpT, er


def kernel(x, qkv_w, q_bias, v_bias, rel_bias_table, proj_w, proj_b, H, W,
           _return_results=False):
    from concourse.bass_utils import run_bass_kernel_spmd

    x = np.asarray(x)
    B = x.shape[0]
    H = int(H)
    W = int(W)
    nws = H // WS
    assert (np.abs(np.asarray(q_bias)).max() == 0
            and np.abs(np.asarray(v_bias)).max() == 0
            and np.abs(np.asarray(proj_b)).max() == 0), \
        "v2 kernel compiled for zero biases (problem spec)"

    xbf, wqkvT, wpT, er = _host_prep(
        x, qkv_w, q_bias, v_bias, rel_bias_table, proj_w, proj_b, H, W)

    Bw = B * nws * nws
    n_win_core = Bw // NCORES
    nc = _get_program(n_win_core)

    tok_core = n_win_core * N
    in_maps = []
    for c in range(NCORES):
        in_maps.append({
            "x": xbf[c * tok_core:(c + 1) * tok_core],
            "wqkvT": wqkvT, "wpT": wpT, "er": er,
        })

    res = run_bass_kernel_spmd(nc, in_maps, list(range(NCORES)))
    # y per core: [C, tok_core] channel-major bf16
    yw = np.concatenate(
        [np.asarray(res.results[c]["y"]).astype(np.float32).T
         for c in range(NCORES)], axis=0)  # [Bw*N, C]
    out = (yw.reshape(B, nws, nws, WS, WS, C)
           .transpose(0, 1, 3, 2, 4, 5)
           .reshape(B, H * W, C))
    if _return_results:
        return out, res
    return out
